# revision 13
# baseline (speedup 1.0000x reference)
"""DFMNET (2-layer LSTM + KDN MLP head) Trainium2 Bass kernel.

Data-parallel over batch: B=2048 split across 8 NeuronCores (256 rows each).
Per-core layout: features on partitions, batch on the free dimension.
  - states: h1,h2 [H=128, B=256] bf16 SBUF (matmul rhs must be SBUF);
    cell states c1,c2 [128, 256] f32 kept in PSUM so tanh(c) reads
    PSUM-source (ScalarE PSUM reads are ~2x faster than SBUF reads)
  - gates computed as gates^T [4H, B] f32 in PSUM via PE matmuls
    (x-part K=64 + h-part K=128 accumulated per gate), weights/x/h in
    bf16 (fp32 matmuls run as two hi/lo passes - 2x slower)
  - gate order repacked host-side to (g, i, f, o): tanh(g) issues first,
    sigmoid(i,f) as one 512-wide op feeds the c-update, sigmoid(o) is
    deferred past the c-update since h = o*tanh(c) needs it last
  - when all biases are zero (true for this model) activations fuse
    wide with no bias APs; otherwise a per-gate bias-AP path is used
  - x transposed host-side into [p=(t%2)*64+i, t//2, b] so it DMAs
    contiguously and serves directly as matmul rhs; W_ih1/Wk0x are
    duplicated into both 64-partition halves so even/odd timesteps hit
    matching partition ranges
"""
import sys

sys.path.insert(0, "/opt/trn_rl_repo")

import numpy as np

B, T, I = 2048, 256, 64
H = 128
K = 128
O = 64
NCORES = 8
BL = B // NCORES  # 256 batch rows per core

F32 = None  # set in _build


def _install_axon_hooks():
    """Provide antenv.axon_hooks (missing in the agent image) so trace=True
    can drive NTFF profiling. Harmless if profiling is never requested."""
    import contextlib
    import ctypes
    import types

    if "antenv.axon_hooks" in sys.modules:
        return
    so_path = "/opt/axon/libaxon_pjrt.so"

    def _make():
        try:
            lib = ctypes.CDLL(so_path)
        except OSError:
            return None
        if not hasattr(lib, "axon_start_nrt_profile"):
            return None
        lib.axon_start_nrt_profile.argtypes = [
            ctypes.POINTER(ctypes.c_int64),
            ctypes.c_size_t,
        ]
        lib.axon_start_nrt_profile.restype = ctypes.c_int64
        lib.axon_stop_nrt_profile.argtypes = [ctypes.c_char_p]
        lib.axon_stop_nrt_profile.restype = ctypes.c_int64

        @contextlib.contextmanager
        def _hook(output_dir, device_ids):
            import jax

            jax.devices()
            if device_ids:
                ids = (ctypes.c_int64 * len(device_ids))(*device_ids)
                rc = lib.axon_start_nrt_profile(ids, len(device_ids))
            else:
                rc = lib.axon_start_nrt_profile(None, 0)
            if rc != 0:
                raise RuntimeError(f"axon_start_nrt_profile rc={rc}")
            try:
                yield
            finally:
                n = lib.axon_stop_nrt_profile(str(output_dir).encode())
                if n < 0:
                    raise RuntimeError(f"axon_stop_nrt_profile rc={n}")

        return _hook

    hook = _make()
    mod = types.ModuleType("antenv.axon_hooks")
    mod.get_axon_ntff_profile_hook = lambda: hook
    mod.set_axon_ntff_profile_hook = lambda h: None
    sys.modules["antenv.axon_hooks"] = mod


_install_axon_hooks()

import ml_dtypes  # noqa: E402
import concourse.bacc as bacc  # noqa: E402
import concourse.tile as tile  # noqa: E402
import concourse.mybir as mybir  # noqa: E402
from concourse.bass_utils import run_bass_kernel_spmd  # noqa: E402

AF = mybir.ActivationFunctionType

# Weight-pack column layout (one [128, WC] f32 array, replicated per core)
C_WIH1 = 0        # [128, 512]  W_ih1T duplicated in rows 0-63 and 64-127
C_WHH1 = 512      # [128, 512]
C_WIH2 = 1024     # [128, 512]
C_WHH2 = 1536     # [128, 512]
C_WK0H = 2048     # [128, 128]
C_WK0X = 2176     # [128, 128]  rows 64-127 hold Wk0[:,128:].T
C_WK = 2304       # 4 x [128, 128]  Wk1..Wk4 transposed
C_WK5 = 2816      # [128, 64]
WC = 2880

NB = 14  # bias pack cols: 0-3 b1(i,f,o,g), 4-7 b2, 8-12 bk0..bk4, 13 bk5


def _build_nc(zero_bias=True):
    nc = bacc.Bacc("TRN2", target_bir_lowering=False, debug=False)
    f32 = mybir.dt.float32
    bf16 = mybir.dt.bfloat16

    xT = nc.dram_tensor("xT", [128, T // 2, BL], bf16, kind="ExternalInput")
    Wp = nc.dram_tensor("Wp", [128, WC], bf16, kind="ExternalInput")
    bp = nc.dram_tensor("bp", [128, NB], f32, kind="ExternalInput")
    yT = nc.dram_tensor("yT", [O, BL], f32, kind="ExternalOutput")
    h2T = nc.dram_tensor("h2T", [H, BL], bf16, kind="ExternalOutput")

    with tile.TileContext(nc) as tc:
        with (
            tc.tile_pool(name="const", bufs=1) as cpool,
            tc.tile_pool(name="state", bufs=1) as spool,
            tc.tile_pool(name="gates", bufs=4) as gpool,
            tc.tile_pool(name="cpsum", bufs=1, space="PSUM") as cppool,
            tc.tile_pool(name="psum", bufs=3, space="PSUM") as ppool,
        ):
            xbig = cpool.tile([128, (T // 2) * BL], bf16, tag="xbig")
            for c in range(8):
                nc.sync.dma_start(
                    xbig[:, c * 16 * BL:(c + 1) * 16 * BL],
                    xT[:, c * 16:(c + 1) * 16, :],
                )
            wp = cpool.tile([128, WC], bf16, tag="wp")
            nc.sync.dma_start(wp[:], Wp[:])
            bpt = cpool.tile([128, NB], f32, tag="bpt")
            nc.sync.dma_start(bpt[:], bp[:])

            h1 = spool.tile([128, BL], bf16, tag="h1")
            h2 = spool.tile([128, BL], bf16, tag="h2")
            c1 = cppool.tile([128, BL], f32, tag="c1")
            c2 = cppool.tile([128, BL], f32, tag="c2")
            nc.vector.memset(h1[:], 0.0)
            nc.vector.memset(h2[:], 0.0)
            nc.vector.memset(c1[:], 0.0)
            nc.vector.memset(c2[:], 0.0)

            # gate order in pack/psum: (g, i, f, o)
            def cell(pg, ccur, hnext, bias_base):
                gg = gpool.tile([128, BL], f32, tag="gg")
                if zero_bias:
                    nc.scalar.activation(gg[:], pg[:, 0:BL], AF.Tanh)
                    if_ = gpool.tile([128, 2 * BL], f32, tag="if_")
                    nc.scalar.activation(if_[:], pg[:, BL:3 * BL], AF.Sigmoid)
                    oot = gpool.tile([128, BL], f32, tag="oo")
                    ii, ff, oo = if_[:, 0:BL], if_[:, BL:2 * BL], oot[:]
                    fc = gpool.tile([128, BL], f32, tag="fc")
                    nc.vector.tensor_mul(fc[:], ff, ccur[:])
                    ig = gpool.tile([128, BL], f32, tag="ig")
                    nc.vector.tensor_mul(ig[:], ii, gg[:])
                    nc.vector.tensor_add(ccur[:], fc[:], ig[:])
                    nc.scalar.activation(oot[:], pg[:, 3 * BL:4 * BL],
                                         AF.Sigmoid)
                    tc_ = gpool.tile([128, BL], bf16, tag="tc_")
                    nc.scalar.activation(tc_[:], ccur[:], AF.Tanh)
                    nc.vector.tensor_mul(hnext[:], oo, tc_[:])
                    return
                iit = gpool.tile([128, BL], f32, tag="ii")
                fft = gpool.tile([128, BL], f32, tag="ff")
                oot = gpool.tile([128, BL], f32, tag="oo")
                nc.scalar.activation(gg[:], pg[:, 0:BL], AF.Tanh,
                                     bias=bpt[:, bias_base:bias_base + 1])
                nc.scalar.activation(iit[:], pg[:, BL:2 * BL], AF.Sigmoid,
                                     bias=bpt[:, bias_base + 1:bias_base + 2])
                nc.scalar.activation(fft[:], pg[:, 2 * BL:3 * BL], AF.Sigmoid,
                                     bias=bpt[:, bias_base + 2:bias_base + 3])
                nc.scalar.activation(oot[:], pg[:, 3 * BL:4 * BL], AF.Sigmoid,
                                     bias=bpt[:, bias_base + 3:bias_base + 4])
                ii, ff, oo = iit[:], fft[:], oot[:]
                fc = gpool.tile([128, BL], f32, tag="fc")
                nc.vector.tensor_mul(fc[:], ff, ccur[:])
                ig = gpool.tile([128, BL], f32, tag="ig")
                nc.vector.tensor_mul(ig[:], ii, gg[:])
                nc.vector.tensor_add(ccur[:], fc[:], ig[:])
                tc_ = gpool.tile([128, BL], bf16, tag="tc_")
                nc.scalar.activation(tc_[:], ccur[:], AF.Tanh)
                nc.vector.tensor_mul(hnext[:], oo, tc_[:])

            for t in range(T):
                tp, t2 = t % 2, t // 2
                xs = xbig[tp * 64:(tp + 1) * 64, t2 * BL:(t2 + 1) * BL]
                wih1 = wp[tp * 64:(tp + 1) * 64, C_WIH1:C_WIH1 + 512]

                pg1 = ppool.tile([128, 4 * BL], f32, tag="pg")
                for j in range(4):
                    seg = pg1[:, j * BL:(j + 1) * BL]
                    nc.tensor.matmul(seg, wih1[:, j * H:(j + 1) * H], xs,
                                     start=True, stop=False)
                    nc.tensor.matmul(
                        seg, wp[:, C_WHH1 + j * H:C_WHH1 + (j + 1) * H], h1[:],
                        start=False, stop=True)
                cell(pg1, h1, c1, h1, 0)

                pg2 = ppool.tile([128, 4 * BL], f32, tag="pg")
                for j in range(4):
                    seg = pg2[:, j * BL:(j + 1) * BL]
                    nc.tensor.matmul(
                        seg, wp[:, C_WIH2 + j * H:C_WIH2 + (j + 1) * H], h1[:],
                        start=True, stop=False)
                    nc.tensor.matmul(
                        seg, wp[:, C_WHH2 + j * H:C_WHH2 + (j + 1) * H], h2[:],
                        start=False, stop=True)
                cell(pg2, h2, c2, h2, 4)

            # ---- KDN head ----
            x_last = xbig[64:128, (T // 2 - 1) * BL:(T // 2) * BL]
            wk0x = wp[64:128, C_WK0X:C_WK0X + 128]
            pk = ppool.tile([128, 4 * BL], f32, tag="pg")
            seg = pk[:, 0:BL]
            nc.tensor.matmul(seg, wp[:, C_WK0H:C_WK0H + 128], h2[:],
                             start=True, stop=False)
            nc.tensor.matmul(seg, wk0x, x_last, start=False, stop=True)
            yk = gpool.tile([128, BL], bf16, tag="yk0")
            if zero_bias:
                nc.scalar.activation(yk[:], seg, AF.Relu)
            else:
                nc.scalar.activation(yk[:], seg, AF.Relu, bias=bpt[:, 8:9])
            for k in range(1, 5):
                pk2 = ppool.tile([128, 4 * BL], f32, tag="pg")
                seg = pk2[:, 0:BL]
                nc.tensor.matmul(
                    seg, wp[:, C_WK + (k - 1) * 128:C_WK + k * 128], yk[:],
                    start=True, stop=True)
                yk2 = gpool.tile([128, BL], bf16, tag=f"yk{k}")
                if zero_bias:
                    nc.scalar.activation(yk2[:], seg, AF.Relu)
                else:
                    nc.scalar.activation(yk2[:], seg, AF.Relu,
                                         bias=bpt[:, 8 + k:9 + k])
                yk = yk2
            pk5 = ppool.tile([128, 4 * BL], f32, tag="pg")
            seg5 = pk5[0:O, 0:BL]
            nc.tensor.matmul(seg5, wp[:, C_WK5:C_WK5 + O], yk[:],
                             start=True, stop=True)
            y5 = gpool.tile([128, BL], f32, tag="y5")
            if zero_bias:
                nc.scalar.copy(y5[0:O, :], seg5)
            else:
                nc.scalar.activation(y5[0:O, :], seg5, AF.Identity,
                                     bias=bpt[0:O, 13:14])

            nc.sync.dma_start(yT[:], y5[0:O, :])
            nc.sync.dma_start(h2T[:], h2[:])

    nc.compile()
    return nc


def _reorder_gates(a):
    """torch gate order (i,f,g,o) -> kernel order (g,o,i,f) along axis 0.

    tanh(g) runs first; sigmoid(i,f) feeds the c-update; sigmoid(o) is
    deferred past the c-update since h needs it last."""
    i, f, g, o = np.split(a, 4, axis=0)
    return np.concatenate([g, i, f, o], axis=0)


def _pack_weights(inp, zero_bias=False):
    def lstm_w(name):
        return _reorder_gates(inp[name]).astype(np.float32).T

    Wp = np.zeros((128, WC), np.float32)
    wih1T = lstm_w("W_ih1")  # [64, 512]
    Wp[0:64, C_WIH1:C_WIH1 + 512] = wih1T
    Wp[64:128, C_WIH1:C_WIH1 + 512] = wih1T
    Wp[:, C_WHH1:C_WHH1 + 512] = lstm_w("W_hh1")
    Wp[:, C_WIH2:C_WIH2 + 512] = lstm_w("W_ih2")
    Wp[:, C_WHH2:C_WHH2 + 512] = lstm_w("W_hh2")
    Wp[:, C_WK0H:C_WK0H + 128] = inp["Wk0"][:, :H].T
    Wp[64:128, C_WK0X:C_WK0X + 128] = inp["Wk0"][:, H:].T
    for k in range(1, 5):
        Wp[:, C_WK + (k - 1) * 128:C_WK + k * 128] = inp[f"Wk{k}"].T
    Wp[:, C_WK5:C_WK5 + O] = inp["Wk5"].T
    bp = np.zeros((128, NB), np.float32)
    b1 = _reorder_gates((inp["b_ih1"] + inp["b_hh1"]).reshape(4 * H, 1))[:, 0]
    b2 = _reorder_gates((inp["b_ih2"] + inp["b_hh2"]).reshape(4 * H, 1))[:, 0]
    for j in range(4):
        bp[:, j] = b1[j * H:(j + 1) * H]
        bp[:, 4 + j] = b2[j * H:(j + 1) * H]
    for k in range(5):
        bp[:, 8 + k] = inp[f"bk{k}"]
    bp[0:O, 13] = inp["bk5"]
    return Wp, bp


_NC_CACHE = {}


def _get_nc(zero_bias):
    key = ("nc", bool(zero_bias))
    if key not in _NC_CACHE:
        _NC_CACHE[key] = _build_nc(zero_bias=zero_bias)
    return _NC_CACHE[key]


def run(inputs, trace=False, tmpdir=None):
    inp = {k: np.asarray(v) for k, v in inputs.items()}
    zero_bias = all(
        not np.any(np.asarray(inp[k]))
        for k in ("b_ih1", "b_hh1", "b_ih2", "b_hh2",
                  "bk0", "bk1", "bk2", "bk3", "bk4", "bk5"))
    Wp, bp = _pack_weights(inp, zero_bias=zero_bias)
    Wpb = Wp.astype(ml_dtypes.bfloat16)
    x = np.asarray(inp["x"], np.float32)
    in_maps = []
    for c in range(NCORES):
        xc = x[c * BL:(c + 1) * BL]                     # [BL, T, I]
        xt = np.ascontiguousarray(xc.transpose(1, 2, 0))  # [T, I, BL]
        xt = np.ascontiguousarray(
            xt.reshape(T // 2, 2, I, BL).transpose(1, 2, 0, 3)
        ).reshape(128, T // 2, BL).astype(ml_dtypes.bfloat16)
        in_maps.append({"xT": xt, "Wp": Wpb, "bp": bp})
    nc = _get_nc(zero_bias)
    r = run_bass_kernel_spmd(nc, in_maps, list(range(NCORES)),
                             trace=trace, tmpdir=tmpdir)
    ys, h2s = [], []
    for c in range(NCORES):
        res = r.results[c]
        ys.append(res["yT"].T)
        h2s.append(res["h2T"].astype(np.float32).T)
    y = np.ascontiguousarray(np.concatenate(ys)).astype(np.float32)
    h2 = np.ascontiguousarray(np.concatenate(h2s)).astype(np.float32)
    rr = np.ascontiguousarray(
        np.concatenate([h2, x[:, T - 1, :]], axis=1)).astype(np.float32)
    return (y, h2, rr), r


def kernel(**inputs):
    out, _ = run(inputs)
    return out


# revision 14
# speedup vs baseline: 1.0001x; 1.0001x over previous
"""DFMNET (2-layer LSTM + KDN MLP head) Trainium2 Bass kernel.

Data-parallel over batch: B=2048 split across 8 NeuronCores (256 rows each).
Per-core layout: features on partitions, batch on the free dimension.
  - states: h1,h2 [H=128, B=256] bf16 SBUF (matmul rhs must be SBUF);
    cell states c1,c2 [128, 256] f32 kept in PSUM so tanh(c) reads
    PSUM-source (ScalarE PSUM reads are ~2x faster than SBUF reads)
  - gates computed as gates^T [4H, B] f32 in PSUM via PE matmuls
    (x-part K=64 + h-part K=128 accumulated per gate), weights/x/h in
    bf16 (fp32 matmuls run as two hi/lo passes - 2x slower)
  - gate order repacked host-side to (g, i, f, o): tanh(g) issues first,
    sigmoid(i,f) as one 512-wide op feeds the c-update, sigmoid(o) is
    deferred past the c-update since h = o*tanh(c) needs it last
  - when all biases are zero (true for this model) activations fuse
    wide with no bias APs; otherwise a per-gate bias-AP path is used
  - x transposed host-side into [p=(t%2)*64+i, t//2, b] so it DMAs
    contiguously and serves directly as matmul rhs; W_ih1/Wk0x are
    duplicated into both 64-partition halves so even/odd timesteps hit
    matching partition ranges
"""
import sys

sys.path.insert(0, "/opt/trn_rl_repo")

import numpy as np

B, T, I = 2048, 256, 64
H = 128
K = 128
O = 64
NCORES = 8
BL = B // NCORES  # 256 batch rows per core

F32 = None  # set in _build


def _install_axon_hooks():
    """Provide antenv.axon_hooks (missing in the agent image) so trace=True
    can drive NTFF profiling. Harmless if profiling is never requested."""
    import contextlib
    import ctypes
    import types

    if "antenv.axon_hooks" in sys.modules:
        return
    so_path = "/opt/axon/libaxon_pjrt.so"

    def _make():
        try:
            lib = ctypes.CDLL(so_path)
        except OSError:
            return None
        if not hasattr(lib, "axon_start_nrt_profile"):
            return None
        lib.axon_start_nrt_profile.argtypes = [
            ctypes.POINTER(ctypes.c_int64),
            ctypes.c_size_t,
        ]
        lib.axon_start_nrt_profile.restype = ctypes.c_int64
        lib.axon_stop_nrt_profile.argtypes = [ctypes.c_char_p]
        lib.axon_stop_nrt_profile.restype = ctypes.c_int64

        @contextlib.contextmanager
        def _hook(output_dir, device_ids):
            import jax

            jax.devices()
            if device_ids:
                ids = (ctypes.c_int64 * len(device_ids))(*device_ids)
                rc = lib.axon_start_nrt_profile(ids, len(device_ids))
            else:
                rc = lib.axon_start_nrt_profile(None, 0)
            if rc != 0:
                raise RuntimeError(f"axon_start_nrt_profile rc={rc}")
            try:
                yield
            finally:
                n = lib.axon_stop_nrt_profile(str(output_dir).encode())
                if n < 0:
                    raise RuntimeError(f"axon_stop_nrt_profile rc={n}")

        return _hook

    hook = _make()
    mod = types.ModuleType("antenv.axon_hooks")
    mod.get_axon_ntff_profile_hook = lambda: hook
    mod.set_axon_ntff_profile_hook = lambda h: None
    sys.modules["antenv.axon_hooks"] = mod


_install_axon_hooks()

import ml_dtypes  # noqa: E402
import concourse.bacc as bacc  # noqa: E402
import concourse.tile as tile  # noqa: E402
import concourse.mybir as mybir  # noqa: E402
from concourse.bass_utils import run_bass_kernel_spmd  # noqa: E402

AF = mybir.ActivationFunctionType

# Weight-pack column layout (one [128, WC] f32 array, replicated per core)
C_WIH1 = 0        # [128, 512]  W_ih1T duplicated in rows 0-63 and 64-127
C_WHH1 = 512      # [128, 512]
C_WIH2 = 1024     # [128, 512]
C_WHH2 = 1536     # [128, 512]
C_WK0H = 2048     # [128, 128]
C_WK0X = 2176     # [128, 128]  rows 64-127 hold Wk0[:,128:].T
C_WK = 2304       # 4 x [128, 128]  Wk1..Wk4 transposed
C_WK5 = 2816      # [128, 64]
WC = 2880

NB = 14  # bias pack cols: 0-3 b1(i,f,o,g), 4-7 b2, 8-12 bk0..bk4, 13 bk5


def _build_nc(zero_bias=True):
    nc = bacc.Bacc("TRN2", target_bir_lowering=False, debug=False)
    f32 = mybir.dt.float32
    bf16 = mybir.dt.bfloat16

    xT = nc.dram_tensor("xT", [128, T // 2, BL], bf16, kind="ExternalInput")
    Wp = nc.dram_tensor("Wp", [128, WC], bf16, kind="ExternalInput")
    bp = nc.dram_tensor("bp", [128, NB], f32, kind="ExternalInput")
    yT = nc.dram_tensor("yT", [O, BL], f32, kind="ExternalOutput")
    h2T = nc.dram_tensor("h2T", [H, BL], bf16, kind="ExternalOutput")

    with tile.TileContext(nc) as tc:
        with (
            tc.tile_pool(name="const", bufs=1) as cpool,
            tc.tile_pool(name="state", bufs=1) as spool,
            tc.tile_pool(name="gates", bufs=3) as gpool,
            tc.tile_pool(name="cpsum", bufs=1, space="PSUM") as cppool,
            tc.tile_pool(name="psum", bufs=3, space="PSUM") as ppool,
        ):
            xbig = cpool.tile([128, (T // 2) * BL], bf16, tag="xbig")
            for c in range(8):
                nc.sync.dma_start(
                    xbig[:, c * 16 * BL:(c + 1) * 16 * BL],
                    xT[:, c * 16:(c + 1) * 16, :],
                )
            wp = cpool.tile([128, WC], bf16, tag="wp")
            nc.sync.dma_start(wp[:], Wp[:])
            bpt = cpool.tile([128, NB], f32, tag="bpt")
            nc.sync.dma_start(bpt[:], bp[:])

            h1 = spool.tile([128, BL], bf16, tag="h1")
            h2 = spool.tile([128, BL], bf16, tag="h2")
            c1 = cppool.tile([128, BL], f32, tag="c1")
            c2 = cppool.tile([128, BL], f32, tag="c2")
            nc.vector.memset(h1[:], 0.0)
            nc.vector.memset(h2[:], 0.0)
            nc.vector.memset(c1[:], 0.0)
            nc.vector.memset(c2[:], 0.0)

            # gate order in pack/psum: (g, i, f, o)
            def cell(pg, ccur, hnext, bias_base):
                gg = gpool.tile([128, BL], f32, tag="gg")
                if zero_bias:
                    nc.scalar.activation(gg[:], pg[:, 0:BL], AF.Tanh)
                    if_ = gpool.tile([128, 2 * BL], f32, tag="if_")
                    nc.scalar.activation(if_[:], pg[:, BL:3 * BL], AF.Sigmoid)
                    oot = gpool.tile([128, BL], f32, tag="oo")
                    ii, ff, oo = if_[:, 0:BL], if_[:, BL:2 * BL], oot[:]
                    fc = gpool.tile([128, BL], f32, tag="fc")
                    nc.vector.tensor_mul(fc[:], ff, ccur[:])
                    ig = gpool.tile([128, BL], f32, tag="ig")
                    nc.vector.tensor_mul(ig[:], ii, gg[:])
                    nc.vector.tensor_add(ccur[:], fc[:], ig[:])
                    nc.scalar.activation(oot[:], pg[:, 3 * BL:4 * BL],
                                         AF.Sigmoid)
                    tc_ = gpool.tile([128, BL], bf16, tag="tc_")
                    nc.scalar.activation(tc_[:], ccur[:], AF.Tanh)
                    nc.vector.tensor_mul(hnext[:], oo, tc_[:])
                    return
                iit = gpool.tile([128, BL], f32, tag="ii")
                fft = gpool.tile([128, BL], f32, tag="ff")
                oot = gpool.tile([128, BL], f32, tag="oo")
                nc.scalar.activation(gg[:], pg[:, 0:BL], AF.Tanh,
                                     bias=bpt[:, bias_base:bias_base + 1])
                nc.scalar.activation(iit[:], pg[:, BL:2 * BL], AF.Sigmoid,
                                     bias=bpt[:, bias_base + 1:bias_base + 2])
                nc.scalar.activation(fft[:], pg[:, 2 * BL:3 * BL], AF.Sigmoid,
                                     bias=bpt[:, bias_base + 2:bias_base + 3])
                nc.scalar.activation(oot[:], pg[:, 3 * BL:4 * BL], AF.Sigmoid,
                                     bias=bpt[:, bias_base + 3:bias_base + 4])
                ii, ff, oo = iit[:], fft[:], oot[:]
                fc = gpool.tile([128, BL], f32, tag="fc")
                nc.vector.tensor_mul(fc[:], ff, ccur[:])
                ig = gpool.tile([128, BL], f32, tag="ig")
                nc.vector.tensor_mul(ig[:], ii, gg[:])
                nc.vector.tensor_add(ccur[:], fc[:], ig[:])
                tc_ = gpool.tile([128, BL], bf16, tag="tc_")
                nc.scalar.activation(tc_[:], ccur[:], AF.Tanh)
                nc.vector.tensor_mul(hnext[:], oo, tc_[:])

            for t in range(T):
                tp, t2 = t % 2, t // 2
                xs = xbig[tp * 64:(tp + 1) * 64, t2 * BL:(t2 + 1) * BL]
                wih1 = wp[tp * 64:(tp + 1) * 64, C_WIH1:C_WIH1 + 512]

                pg1 = ppool.tile([128, 4 * BL], f32, tag="pg")
                for j in range(4):
                    seg = pg1[:, j * BL:(j + 1) * BL]
                    nc.tensor.matmul(seg, wih1[:, j * H:(j + 1) * H], xs,
                                     start=True, stop=False)
                    nc.tensor.matmul(
                        seg, wp[:, C_WHH1 + j * H:C_WHH1 + (j + 1) * H], h1[:],
                        start=False, stop=True)
                cell(pg1, h1, c1, h1, 0)

                pg2 = ppool.tile([128, 4 * BL], f32, tag="pg")
                for j in range(4):
                    seg = pg2[:, j * BL:(j + 1) * BL]
                    nc.tensor.matmul(
                        seg, wp[:, C_WIH2 + j * H:C_WIH2 + (j + 1) * H], h1[:],
                        start=True, stop=False)
                    nc.tensor.matmul(
                        seg, wp[:, C_WHH2 + j * H:C_WHH2 + (j + 1) * H], h2[:],
                        start=False, stop=True)
                cell(pg2, h2, c2, h2, 4)

            # ---- KDN head ----
            x_last = xbig[64:128, (T // 2 - 1) * BL:(T // 2) * BL]
            wk0x = wp[64:128, C_WK0X:C_WK0X + 128]
            pk = ppool.tile([128, 4 * BL], f32, tag="pg")
            seg = pk[:, 0:BL]
            nc.tensor.matmul(seg, wp[:, C_WK0H:C_WK0H + 128], h2[:],
                             start=True, stop=False)
            nc.tensor.matmul(seg, wk0x, x_last, start=False, stop=True)
            yk = gpool.tile([128, BL], bf16, tag="yk0")
            if zero_bias:
                nc.scalar.activation(yk[:], seg, AF.Relu)
            else:
                nc.scalar.activation(yk[:], seg, AF.Relu, bias=bpt[:, 8:9])
            for k in range(1, 5):
                pk2 = ppool.tile([128, 4 * BL], f32, tag="pg")
                seg = pk2[:, 0:BL]
                nc.tensor.matmul(
                    seg, wp[:, C_WK + (k - 1) * 128:C_WK + k * 128], yk[:],
                    start=True, stop=True)
                yk2 = gpool.tile([128, BL], bf16, tag=f"yk{k}")
                if zero_bias:
                    nc.scalar.activation(yk2[:], seg, AF.Relu)
                else:
                    nc.scalar.activation(yk2[:], seg, AF.Relu,
                                         bias=bpt[:, 8 + k:9 + k])
                yk = yk2
            pk5 = ppool.tile([128, 4 * BL], f32, tag="pg")
            seg5 = pk5[0:O, 0:BL]
            nc.tensor.matmul(seg5, wp[:, C_WK5:C_WK5 + O], yk[:],
                             start=True, stop=True)
            y5 = gpool.tile([128, BL], f32, tag="y5")
            if zero_bias:
                nc.scalar.copy(y5[0:O, :], seg5)
            else:
                nc.scalar.activation(y5[0:O, :], seg5, AF.Identity,
                                     bias=bpt[0:O, 13:14])

            nc.sync.dma_start(yT[:], y5[0:O, :])
            nc.sync.dma_start(h2T[:], h2[:])

    nc.compile()
    return nc


def _reorder_gates(a):
    """torch gate order (i,f,g,o) -> kernel order (g,o,i,f) along axis 0.

    tanh(g) runs first; sigmoid(i,f) feeds the c-update; sigmoid(o) is
    deferred past the c-update since h needs it last."""
    i, f, g, o = np.split(a, 4, axis=0)
    return np.concatenate([g, i, f, o], axis=0)


def _pack_weights(inp, zero_bias=False):
    def lstm_w(name):
        return _reorder_gates(inp[name]).astype(np.float32).T

    Wp = np.zeros((128, WC), np.float32)
    wih1T = lstm_w("W_ih1")  # [64, 512]
    Wp[0:64, C_WIH1:C_WIH1 + 512] = wih1T
    Wp[64:128, C_WIH1:C_WIH1 + 512] = wih1T
    Wp[:, C_WHH1:C_WHH1 + 512] = lstm_w("W_hh1")
    Wp[:, C_WIH2:C_WIH2 + 512] = lstm_w("W_ih2")
    Wp[:, C_WHH2:C_WHH2 + 512] = lstm_w("W_hh2")
    Wp[:, C_WK0H:C_WK0H + 128] = inp["Wk0"][:, :H].T
    Wp[64:128, C_WK0X:C_WK0X + 128] = inp["Wk0"][:, H:].T
    for k in range(1, 5):
        Wp[:, C_WK + (k - 1) * 128:C_WK + k * 128] = inp[f"Wk{k}"].T
    Wp[:, C_WK5:C_WK5 + O] = inp["Wk5"].T
    bp = np.zeros((128, NB), np.float32)
    b1 = _reorder_gates((inp["b_ih1"] + inp["b_hh1"]).reshape(4 * H, 1))[:, 0]
    b2 = _reorder_gates((inp["b_ih2"] + inp["b_hh2"]).reshape(4 * H, 1))[:, 0]
    for j in range(4):
        bp[:, j] = b1[j * H:(j + 1) * H]
        bp[:, 4 + j] = b2[j * H:(j + 1) * H]
    for k in range(5):
        bp[:, 8 + k] = inp[f"bk{k}"]
    bp[0:O, 13] = inp["bk5"]
    return Wp, bp


_NC_CACHE = {}


def _get_nc(zero_bias):
    key = ("nc", bool(zero_bias))
    if key not in _NC_CACHE:
        _NC_CACHE[key] = _build_nc(zero_bias=zero_bias)
    return _NC_CACHE[key]


def run(inputs, trace=False, tmpdir=None):
    inp = {k: np.asarray(v) for k, v in inputs.items()}
    zero_bias = all(
        not np.any(np.asarray(inp[k]))
        for k in ("b_ih1", "b_hh1", "b_ih2", "b_hh2",
                  "bk0", "bk1", "bk2", "bk3", "bk4", "bk5"))
    Wp, bp = _pack_weights(inp, zero_bias=zero_bias)
    Wpb = Wp.astype(ml_dtypes.bfloat16)
    x = np.asarray(inp["x"], np.float32)
    in_maps = []
    for c in range(NCORES):
        xc = x[c * BL:(c + 1) * BL]                     # [BL, T, I]
        xt = np.ascontiguousarray(xc.transpose(1, 2, 0))  # [T, I, BL]
        xt = np.ascontiguousarray(
            xt.reshape(T // 2, 2, I, BL).transpose(1, 2, 0, 3)
        ).reshape(128, T // 2, BL).astype(ml_dtypes.bfloat16)
        in_maps.append({"xT": xt, "Wp": Wpb, "bp": bp})
    nc = _get_nc(zero_bias)
    r = run_bass_kernel_spmd(nc, in_maps, list(range(NCORES)),
                             trace=trace, tmpdir=tmpdir)
    ys, h2s = [], []
    for c in range(NCORES):
        res = r.results[c]
        ys.append(res["yT"].T)
        h2s.append(res["h2T"].astype(np.float32).T)
    y = np.ascontiguousarray(np.concatenate(ys)).astype(np.float32)
    h2 = np.ascontiguousarray(np.concatenate(h2s)).astype(np.float32)
    rr = np.ascontiguousarray(
        np.concatenate([h2, x[:, T - 1, :]], axis=1)).astype(np.float32)
    return (y, h2, rr), r


def kernel(**inputs):
    out, _ = run(inputs)
    return out


# revision 15
# speedup vs baseline: 1.0035x; 1.0034x over previous
"""DFMNET (2-layer LSTM + KDN MLP head) Trainium2 Bass kernel.

Data-parallel over batch: B=2048 split across 8 NeuronCores (256 rows each).
Per-core layout: features on partitions, batch on the free dimension.
  - states: h1,h2 [H=128, B=256] bf16 SBUF (matmul rhs must be SBUF);
    cell states c1,c2 [128, 256] f32 kept in PSUM so tanh(c) reads
    PSUM-source (ScalarE PSUM reads are ~2x faster than SBUF reads)
  - gates computed as gates^T [4H, B] f32 in PSUM via PE matmuls
    (x-part K=64 + h-part K=128 accumulated per gate), weights/x/h in
    bf16 (fp32 matmuls run as two hi/lo passes - 2x slower)
  - gate order repacked host-side to (g, i, f, o): tanh(g) issues first,
    sigmoid(i,f) as one 512-wide op feeds the c-update, sigmoid(o) is
    deferred past the c-update since h = o*tanh(c) needs it last
  - when all biases are zero (true for this model) activations fuse
    wide with no bias APs; otherwise a per-gate bias-AP path is used
  - x transposed host-side into [p=(t%2)*64+i, t//2, b] so it DMAs
    contiguously and serves directly as matmul rhs; W_ih1/Wk0x are
    duplicated into both 64-partition halves so even/odd timesteps hit
    matching partition ranges
"""
import sys

sys.path.insert(0, "/opt/trn_rl_repo")

import numpy as np

B, T, I = 2048, 256, 64
H = 128
K = 128
O = 64
NCORES = 8
BL = B // NCORES  # 256 batch rows per core

F32 = None  # set in _build


def _install_axon_hooks():
    """Provide antenv.axon_hooks (missing in the agent image) so trace=True
    can drive NTFF profiling. Harmless if profiling is never requested."""
    import contextlib
    import ctypes
    import types

    if "antenv.axon_hooks" in sys.modules:
        return
    so_path = "/opt/axon/libaxon_pjrt.so"

    def _make():
        try:
            lib = ctypes.CDLL(so_path)
        except OSError:
            return None
        if not hasattr(lib, "axon_start_nrt_profile"):
            return None
        lib.axon_start_nrt_profile.argtypes = [
            ctypes.POINTER(ctypes.c_int64),
            ctypes.c_size_t,
        ]
        lib.axon_start_nrt_profile.restype = ctypes.c_int64
        lib.axon_stop_nrt_profile.argtypes = [ctypes.c_char_p]
        lib.axon_stop_nrt_profile.restype = ctypes.c_int64

        @contextlib.contextmanager
        def _hook(output_dir, device_ids):
            import jax

            jax.devices()
            if device_ids:
                ids = (ctypes.c_int64 * len(device_ids))(*device_ids)
                rc = lib.axon_start_nrt_profile(ids, len(device_ids))
            else:
                rc = lib.axon_start_nrt_profile(None, 0)
            if rc != 0:
                raise RuntimeError(f"axon_start_nrt_profile rc={rc}")
            try:
                yield
            finally:
                n = lib.axon_stop_nrt_profile(str(output_dir).encode())
                if n < 0:
                    raise RuntimeError(f"axon_stop_nrt_profile rc={n}")

        return _hook

    hook = _make()
    mod = types.ModuleType("antenv.axon_hooks")
    mod.get_axon_ntff_profile_hook = lambda: hook
    mod.set_axon_ntff_profile_hook = lambda h: None
    sys.modules["antenv.axon_hooks"] = mod


_install_axon_hooks()

import ml_dtypes  # noqa: E402
import concourse.bacc as bacc  # noqa: E402
import concourse.tile as tile  # noqa: E402
import concourse.mybir as mybir  # noqa: E402
from concourse.bass_utils import run_bass_kernel_spmd  # noqa: E402

AF = mybir.ActivationFunctionType

# Weight-pack column layout (one [128, WC] f32 array, replicated per core)
C_WIH1 = 0        # [128, 512]  W_ih1T duplicated in rows 0-63 and 64-127
C_WHH1 = 512      # [128, 512]
C_WIH2 = 1024     # [128, 512]
C_WHH2 = 1536     # [128, 512]
C_WK0H = 2048     # [128, 128]
C_WK0X = 2176     # [128, 128]  rows 64-127 hold Wk0[:,128:].T
C_WK = 2304       # 4 x [128, 128]  Wk1..Wk4 transposed
C_WK5 = 2816      # [128, 64]
WC = 2880

NB = 14  # bias pack cols: 0-3 b1(i,f,o,g), 4-7 b2, 8-12 bk0..bk4, 13 bk5


def _build_nc(zero_bias=True):
    nc = bacc.Bacc("TRN2", target_bir_lowering=False, debug=False)
    f32 = mybir.dt.float32
    bf16 = mybir.dt.bfloat16

    xT = nc.dram_tensor("xT", [128, T // 2, BL], bf16, kind="ExternalInput")
    Wp = nc.dram_tensor("Wp", [128, WC], bf16, kind="ExternalInput")
    bp = nc.dram_tensor("bp", [128, NB], f32, kind="ExternalInput")
    yT = nc.dram_tensor("yT", [O, BL], f32, kind="ExternalOutput")
    h2T = nc.dram_tensor("h2T", [H, BL], bf16, kind="ExternalOutput")

    with tile.TileContext(nc) as tc:
        with (
            tc.tile_pool(name="const", bufs=1) as cpool,
            tc.tile_pool(name="state", bufs=1) as spool,
            tc.tile_pool(name="gates", bufs=3) as gpool,
            tc.tile_pool(name="cpsum", bufs=1, space="PSUM") as cppool,
            tc.tile_pool(name="psum", bufs=3, space="PSUM") as ppool,
        ):
            xbig = cpool.tile([128, (T // 2) * BL], bf16, tag="xbig")
            for c in range(8):
                nc.sync.dma_start(
                    xbig[:, c * 16 * BL:(c + 1) * 16 * BL],
                    xT[:, c * 16:(c + 1) * 16, :],
                )
            wp = cpool.tile([128, WC], bf16, tag="wp")
            nc.sync.dma_start(wp[:], Wp[:])
            bpt = cpool.tile([128, NB], f32, tag="bpt")
            nc.sync.dma_start(bpt[:], bp[:])

            h1 = spool.tile([128, BL], bf16, tag="h1")
            h2 = spool.tile([128, BL], bf16, tag="h2")
            c1 = cppool.tile([128, BL], f32, tag="c1")
            c2 = cppool.tile([128, BL], f32, tag="c2")
            nc.vector.memset(h1[:], 0.0)
            nc.vector.memset(h2[:], 0.0)
            nc.vector.memset(c1[:], 0.0)
            nc.vector.memset(c2[:], 0.0)

            # gate order in pack/psum: (g, i, f, o)
            def cell(pg, ccur, hnext, bias_base):
                gg = gpool.tile([128, BL], f32, tag="gg")
                if zero_bias:
                    nc.scalar.activation(gg[:], pg[:, 0:BL], AF.Tanh)
                    if_ = gpool.tile([128, 2 * BL], f32, tag="if_")
                    nc.scalar.activation(if_[:], pg[:, BL:3 * BL], AF.Sigmoid)
                    oot = gpool.tile([128, BL], f32, tag="oo")
                    ii, ff, oo = if_[:, 0:BL], if_[:, BL:2 * BL], oot[:]
                    # sig_o directly after sig_if: it is the gate tile's
                    # last reader, so this releases the psum banks ~1.3us
                    # earlier for the next tiles' state-independent matmuls
                    nc.scalar.activation(oot[:], pg[:, 3 * BL:4 * BL],
                                         AF.Sigmoid)
                    fc = gpool.tile([128, BL], f32, tag="fc")
                    nc.vector.tensor_mul(fc[:], ff, ccur[:])
                    ig = gpool.tile([128, BL], f32, tag="ig")
                    nc.vector.tensor_mul(ig[:], ii, gg[:])
                    nc.vector.tensor_add(ccur[:], fc[:], ig[:])
                    tc_ = gpool.tile([128, BL], bf16, tag="tc_")
                    nc.scalar.activation(tc_[:], ccur[:], AF.Tanh)
                    nc.vector.tensor_mul(hnext[:], oo, tc_[:])
                    return
                iit = gpool.tile([128, BL], f32, tag="ii")
                fft = gpool.tile([128, BL], f32, tag="ff")
                oot = gpool.tile([128, BL], f32, tag="oo")
                nc.scalar.activation(gg[:], pg[:, 0:BL], AF.Tanh,
                                     bias=bpt[:, bias_base:bias_base + 1])
                nc.scalar.activation(iit[:], pg[:, BL:2 * BL], AF.Sigmoid,
                                     bias=bpt[:, bias_base + 1:bias_base + 2])
                nc.scalar.activation(fft[:], pg[:, 2 * BL:3 * BL], AF.Sigmoid,
                                     bias=bpt[:, bias_base + 2:bias_base + 3])
                nc.scalar.activation(oot[:], pg[:, 3 * BL:4 * BL], AF.Sigmoid,
                                     bias=bpt[:, bias_base + 3:bias_base + 4])
                ii, ff, oo = iit[:], fft[:], oot[:]
                fc = gpool.tile([128, BL], f32, tag="fc")
                nc.vector.tensor_mul(fc[:], ff, ccur[:])
                ig = gpool.tile([128, BL], f32, tag="ig")
                nc.vector.tensor_mul(ig[:], ii, gg[:])
                nc.vector.tensor_add(ccur[:], fc[:], ig[:])
                tc_ = gpool.tile([128, BL], bf16, tag="tc_")
                nc.scalar.activation(tc_[:], ccur[:], AF.Tanh)
                nc.vector.tensor_mul(hnext[:], oo, tc_[:])

            for t in range(T):
                tp, t2 = t % 2, t // 2
                xs = xbig[tp * 64:(tp + 1) * 64, t2 * BL:(t2 + 1) * BL]
                wih1 = wp[tp * 64:(tp + 1) * 64, C_WIH1:C_WIH1 + 512]

                pg1 = ppool.tile([128, 4 * BL], f32, tag="pg")
                for j in range(4):
                    seg = pg1[:, j * BL:(j + 1) * BL]
                    nc.tensor.matmul(seg, wih1[:, j * H:(j + 1) * H], xs,
                                     start=True, stop=False)
                    nc.tensor.matmul(
                        seg, wp[:, C_WHH1 + j * H:C_WHH1 + (j + 1) * H], h1[:],
                        start=False, stop=True)
                cell(pg1, h1, c1, h1, 0)

                pg2 = ppool.tile([128, 4 * BL], f32, tag="pg")
                for j in range(4):
                    seg = pg2[:, j * BL:(j + 1) * BL]
                    nc.tensor.matmul(
                        seg, wp[:, C_WIH2 + j * H:C_WIH2 + (j + 1) * H], h1[:],
                        start=True, stop=False)
                    nc.tensor.matmul(
                        seg, wp[:, C_WHH2 + j * H:C_WHH2 + (j + 1) * H], h2[:],
                        start=False, stop=True)
                cell(pg2, h2, c2, h2, 4)

            # ---- KDN head ----
            x_last = xbig[64:128, (T // 2 - 1) * BL:(T // 2) * BL]
            wk0x = wp[64:128, C_WK0X:C_WK0X + 128]
            pk = ppool.tile([128, 4 * BL], f32, tag="pg")
            seg = pk[:, 0:BL]
            nc.tensor.matmul(seg, wp[:, C_WK0H:C_WK0H + 128], h2[:],
                             start=True, stop=False)
            nc.tensor.matmul(seg, wk0x, x_last, start=False, stop=True)
            yk = gpool.tile([128, BL], bf16, tag="yk0")
            if zero_bias:
                nc.scalar.activation(yk[:], seg, AF.Relu)
            else:
                nc.scalar.activation(yk[:], seg, AF.Relu, bias=bpt[:, 8:9])
            for k in range(1, 5):
                pk2 = ppool.tile([128, 4 * BL], f32, tag="pg")
                seg = pk2[:, 0:BL]
                nc.tensor.matmul(
                    seg, wp[:, C_WK + (k - 1) * 128:C_WK + k * 128], yk[:],
                    start=True, stop=True)
                yk2 = gpool.tile([128, BL], bf16, tag=f"yk{k}")
                if zero_bias:
                    nc.scalar.activation(yk2[:], seg, AF.Relu)
                else:
                    nc.scalar.activation(yk2[:], seg, AF.Relu,
                                         bias=bpt[:, 8 + k:9 + k])
                yk = yk2
            pk5 = ppool.tile([128, 4 * BL], f32, tag="pg")
            seg5 = pk5[0:O, 0:BL]
            nc.tensor.matmul(seg5, wp[:, C_WK5:C_WK5 + O], yk[:],
                             start=True, stop=True)
            y5 = gpool.tile([128, BL], f32, tag="y5")
            if zero_bias:
                nc.scalar.copy(y5[0:O, :], seg5)
            else:
                nc.scalar.activation(y5[0:O, :], seg5, AF.Identity,
                                     bias=bpt[0:O, 13:14])

            nc.sync.dma_start(yT[:], y5[0:O, :])
            nc.sync.dma_start(h2T[:], h2[:])

    nc.compile()
    return nc


def _reorder_gates(a):
    """torch gate order (i,f,g,o) -> kernel order (g,o,i,f) along axis 0.

    tanh(g) runs first; sigmoid(i,f) feeds the c-update; sigmoid(o) is
    deferred past the c-update since h needs it last."""
    i, f, g, o = np.split(a, 4, axis=0)
    return np.concatenate([g, i, f, o], axis=0)


def _pack_weights(inp, zero_bias=False):
    def lstm_w(name):
        return _reorder_gates(inp[name]).astype(np.float32).T

    Wp = np.zeros((128, WC), np.float32)
    wih1T = lstm_w("W_ih1")  # [64, 512]
    Wp[0:64, C_WIH1:C_WIH1 + 512] = wih1T
    Wp[64:128, C_WIH1:C_WIH1 + 512] = wih1T
    Wp[:, C_WHH1:C_WHH1 + 512] = lstm_w("W_hh1")
    Wp[:, C_WIH2:C_WIH2 + 512] = lstm_w("W_ih2")
    Wp[:, C_WHH2:C_WHH2 + 512] = lstm_w("W_hh2")
    Wp[:, C_WK0H:C_WK0H + 128] = inp["Wk0"][:, :H].T
    Wp[64:128, C_WK0X:C_WK0X + 128] = inp["Wk0"][:, H:].T
    for k in range(1, 5):
        Wp[:, C_WK + (k - 1) * 128:C_WK + k * 128] = inp[f"Wk{k}"].T
    Wp[:, C_WK5:C_WK5 + O] = inp["Wk5"].T
    bp = np.zeros((128, NB), np.float32)
    b1 = _reorder_gates((inp["b_ih1"] + inp["b_hh1"]).reshape(4 * H, 1))[:, 0]
    b2 = _reorder_gates((inp["b_ih2"] + inp["b_hh2"]).reshape(4 * H, 1))[:, 0]
    for j in range(4):
        bp[:, j] = b1[j * H:(j + 1) * H]
        bp[:, 4 + j] = b2[j * H:(j + 1) * H]
    for k in range(5):
        bp[:, 8 + k] = inp[f"bk{k}"]
    bp[0:O, 13] = inp["bk5"]
    return Wp, bp


_NC_CACHE = {}


def _get_nc(zero_bias):
    key = ("nc", bool(zero_bias))
    if key not in _NC_CACHE:
        _NC_CACHE[key] = _build_nc(zero_bias=zero_bias)
    return _NC_CACHE[key]


def run(inputs, trace=False, tmpdir=None):
    inp = {k: np.asarray(v) for k, v in inputs.items()}
    zero_bias = all(
        not np.any(np.asarray(inp[k]))
        for k in ("b_ih1", "b_hh1", "b_ih2", "b_hh2",
                  "bk0", "bk1", "bk2", "bk3", "bk4", "bk5"))
    Wp, bp = _pack_weights(inp, zero_bias=zero_bias)
    Wpb = Wp.astype(ml_dtypes.bfloat16)
    x = np.asarray(inp["x"], np.float32)
    in_maps = []
    for c in range(NCORES):
        xc = x[c * BL:(c + 1) * BL]                     # [BL, T, I]
        xt = np.ascontiguousarray(xc.transpose(1, 2, 0))  # [T, I, BL]
        xt = np.ascontiguousarray(
            xt.reshape(T // 2, 2, I, BL).transpose(1, 2, 0, 3)
        ).reshape(128, T // 2, BL).astype(ml_dtypes.bfloat16)
        in_maps.append({"xT": xt, "Wp": Wpb, "bp": bp})
    nc = _get_nc(zero_bias)
    r = run_bass_kernel_spmd(nc, in_maps, list(range(NCORES)),
                             trace=trace, tmpdir=tmpdir)
    ys, h2s = [], []
    for c in range(NCORES):
        res = r.results[c]
        ys.append(res["yT"].T)
        h2s.append(res["h2T"].astype(np.float32).T)
    y = np.ascontiguousarray(np.concatenate(ys)).astype(np.float32)
    h2 = np.ascontiguousarray(np.concatenate(h2s)).astype(np.float32)
    rr = np.ascontiguousarray(
        np.concatenate([h2, x[:, T - 1, :]], axis=1)).astype(np.float32)
    return (y, h2, rr), r


def kernel(**inputs):
    out, _ = run(inputs)
    return out


# revision 16
# speedup vs baseline: 1.0062x; 1.0027x over previous
"""DFMNET (2-layer LSTM + KDN MLP head) Trainium2 Bass kernel.

Data-parallel over batch: B=2048 split across 8 NeuronCores (256 rows each).
Per-core layout: features on partitions, batch on the free dimension.
  - states: h1,h2 [H=128, B=256] bf16 SBUF (matmul rhs must be SBUF);
    cell states c1,c2 [128, 256] f32 kept in PSUM so tanh(c) reads
    PSUM-source (ScalarE PSUM reads are ~2x faster than SBUF reads)
  - gates computed as gates^T [4H, B] f32 in PSUM via PE matmuls
    (x-part K=64 + h-part K=128 accumulated per gate), weights/x/h in
    bf16 (fp32 matmuls run as two hi/lo passes - 2x slower)
  - gate order repacked host-side to (g, i, f, o): tanh(g) issues first,
    sigmoid(i,f) as one 512-wide op feeds the c-update, sigmoid(o) is
    deferred past the c-update since h = o*tanh(c) needs it last
  - when all biases are zero (true for this model) activations fuse
    wide with no bias APs; otherwise a per-gate bias-AP path is used
  - x transposed host-side into [p=(t%2)*64+i, t//2, b] so it DMAs
    contiguously and serves directly as matmul rhs; W_ih1/Wk0x are
    duplicated into both 64-partition halves so even/odd timesteps hit
    matching partition ranges
"""
import sys

sys.path.insert(0, "/opt/trn_rl_repo")

import numpy as np

B, T, I = 2048, 256, 64
H = 128
K = 128
O = 64
NCORES = 8
BL = B // NCORES  # 256 batch rows per core

F32 = None  # set in _build


def _install_axon_hooks():
    """Provide antenv.axon_hooks (missing in the agent image) so trace=True
    can drive NTFF profiling. Harmless if profiling is never requested."""
    import contextlib
    import ctypes
    import types

    if "antenv.axon_hooks" in sys.modules:
        return
    so_path = "/opt/axon/libaxon_pjrt.so"

    def _make():
        try:
            lib = ctypes.CDLL(so_path)
        except OSError:
            return None
        if not hasattr(lib, "axon_start_nrt_profile"):
            return None
        lib.axon_start_nrt_profile.argtypes = [
            ctypes.POINTER(ctypes.c_int64),
            ctypes.c_size_t,
        ]
        lib.axon_start_nrt_profile.restype = ctypes.c_int64
        lib.axon_stop_nrt_profile.argtypes = [ctypes.c_char_p]
        lib.axon_stop_nrt_profile.restype = ctypes.c_int64

        @contextlib.contextmanager
        def _hook(output_dir, device_ids):
            import jax

            jax.devices()
            if device_ids:
                ids = (ctypes.c_int64 * len(device_ids))(*device_ids)
                rc = lib.axon_start_nrt_profile(ids, len(device_ids))
            else:
                rc = lib.axon_start_nrt_profile(None, 0)
            if rc != 0:
                raise RuntimeError(f"axon_start_nrt_profile rc={rc}")
            try:
                yield
            finally:
                n = lib.axon_stop_nrt_profile(str(output_dir).encode())
                if n < 0:
                    raise RuntimeError(f"axon_stop_nrt_profile rc={n}")

        return _hook

    hook = _make()
    mod = types.ModuleType("antenv.axon_hooks")
    mod.get_axon_ntff_profile_hook = lambda: hook
    mod.set_axon_ntff_profile_hook = lambda h: None
    sys.modules["antenv.axon_hooks"] = mod


_install_axon_hooks()

import ml_dtypes  # noqa: E402
import concourse.bacc as bacc  # noqa: E402
import concourse.tile as tile  # noqa: E402
import concourse.mybir as mybir  # noqa: E402
from concourse.bass_utils import run_bass_kernel_spmd  # noqa: E402

AF = mybir.ActivationFunctionType

# Weight-pack column layout (one [128, WC] f32 array, replicated per core)
C_WIH1 = 0        # [128, 512]  W_ih1T duplicated in rows 0-63 and 64-127
C_WHH1 = 512      # [128, 512]
C_WIH2 = 1024     # [128, 512]
C_WHH2 = 1536     # [128, 512]
C_WK0H = 2048     # [128, 128]
C_WK0X = 2176     # [128, 128]  rows 64-127 hold Wk0[:,128:].T
C_WK = 2304       # 4 x [128, 128]  Wk1..Wk4 transposed
C_WK5 = 2816      # [128, 64]
WC = 2880

NB = 14  # bias pack cols: 0-3 b1(i,f,o,g), 4-7 b2, 8-12 bk0..bk4, 13 bk5


def _build_nc(zero_bias=True):
    nc = bacc.Bacc("TRN2", target_bir_lowering=False, debug=False)
    f32 = mybir.dt.float32
    bf16 = mybir.dt.bfloat16

    xT = nc.dram_tensor("xT", [128, T // 2, BL], bf16, kind="ExternalInput")
    Wp = nc.dram_tensor("Wp", [128, WC], bf16, kind="ExternalInput")
    bp = nc.dram_tensor("bp", [128, NB], f32, kind="ExternalInput")
    yT = nc.dram_tensor("yT", [O, BL], f32, kind="ExternalOutput")
    h2T = nc.dram_tensor("h2T", [H, BL], bf16, kind="ExternalOutput")

    with tile.TileContext(nc) as tc:
        with (
            tc.tile_pool(name="const", bufs=1) as cpool,
            tc.tile_pool(name="state", bufs=1) as spool,
            tc.tile_pool(name="gates", bufs=3) as gpool,
            tc.tile_pool(name="cpsum", bufs=1, space="PSUM") as cppool,
            tc.tile_pool(name="psum", bufs=3, space="PSUM") as ppool,
        ):
            xbig = cpool.tile([128, (T // 2) * BL], bf16, tag="xbig")
            for c in range(8):
                nc.sync.dma_start(
                    xbig[:, c * 16 * BL:(c + 1) * 16 * BL],
                    xT[:, c * 16:(c + 1) * 16, :],
                )
            wp = cpool.tile([128, WC], bf16, tag="wp")
            nc.sync.dma_start(wp[:], Wp[:])
            bpt = cpool.tile([128, NB], f32, tag="bpt")
            nc.sync.dma_start(bpt[:], bp[:])

            h1 = spool.tile([128, BL], bf16, tag="h1")
            h2 = spool.tile([128, BL], bf16, tag="h2")
            c1 = cppool.tile([128, BL], f32, tag="c1")
            c2 = cppool.tile([128, BL], f32, tag="c2")
            nc.vector.memset(h1[:], 0.0)
            nc.vector.memset(h2[:], 0.0)
            nc.vector.memset(c1[:], 0.0)
            nc.vector.memset(c2[:], 0.0)

            # gate order in pack/psum: (g, i, f, o)
            def cell(pg, ccur, hnext, bias_base):
                gg = gpool.tile([128, BL], f32, tag="gg")
                if zero_bias:
                    # gate-g weights pre-scaled x2 host-side: all 4 gates
                    # take sigmoid in ONE 1024-wide ACT (tanh(x) =
                    # 2*sigmoid(2x)-1), g fixed up via DVE tensor_scalar
                    sg = gpool.tile([128, 4 * BL], f32, tag="sg")
                    nc.scalar.activation(sg[:], pg[:, 0:4 * BL], AF.Sigmoid)
                    nc.vector.tensor_scalar(
                        gg[:], sg[:, 0:BL], 2.0, 1.0,
                        mybir.AluOpType.mult, mybir.AluOpType.subtract)
                    ii, ff, oo = (sg[:, BL:2 * BL], sg[:, 2 * BL:3 * BL],
                                  sg[:, 3 * BL:4 * BL])
                    fc = gpool.tile([128, BL], f32, tag="fc")
                    nc.vector.tensor_mul(fc[:], ff, ccur[:])
                    ig = gpool.tile([128, BL], f32, tag="ig")
                    nc.vector.tensor_mul(ig[:], ii, gg[:])
                    nc.vector.tensor_add(ccur[:], fc[:], ig[:])
                    tc_ = gpool.tile([128, BL], bf16, tag="tc_")
                    nc.scalar.activation(tc_[:], ccur[:], AF.Tanh)
                    nc.vector.tensor_mul(hnext[:], oo, tc_[:])
                    return
                iit = gpool.tile([128, BL], f32, tag="ii")
                fft = gpool.tile([128, BL], f32, tag="ff")
                oot = gpool.tile([128, BL], f32, tag="oo")
                nc.scalar.activation(gg[:], pg[:, 0:BL], AF.Tanh,
                                     bias=bpt[:, bias_base:bias_base + 1])
                nc.scalar.activation(iit[:], pg[:, BL:2 * BL], AF.Sigmoid,
                                     bias=bpt[:, bias_base + 1:bias_base + 2])
                nc.scalar.activation(fft[:], pg[:, 2 * BL:3 * BL], AF.Sigmoid,
                                     bias=bpt[:, bias_base + 2:bias_base + 3])
                nc.scalar.activation(oot[:], pg[:, 3 * BL:4 * BL], AF.Sigmoid,
                                     bias=bpt[:, bias_base + 3:bias_base + 4])
                ii, ff, oo = iit[:], fft[:], oot[:]
                fc = gpool.tile([128, BL], f32, tag="fc")
                nc.vector.tensor_mul(fc[:], ff, ccur[:])
                ig = gpool.tile([128, BL], f32, tag="ig")
                nc.vector.tensor_mul(ig[:], ii, gg[:])
                nc.vector.tensor_add(ccur[:], fc[:], ig[:])
                tc_ = gpool.tile([128, BL], bf16, tag="tc_")
                nc.scalar.activation(tc_[:], ccur[:], AF.Tanh)
                nc.vector.tensor_mul(hnext[:], oo, tc_[:])

            for t in range(T):
                tp, t2 = t % 2, t // 2
                xs = xbig[tp * 64:(tp + 1) * 64, t2 * BL:(t2 + 1) * BL]
                wih1 = wp[tp * 64:(tp + 1) * 64, C_WIH1:C_WIH1 + 512]

                pg1 = ppool.tile([128, 4 * BL], f32, tag="pg")
                for j in range(4):
                    seg = pg1[:, j * BL:(j + 1) * BL]
                    nc.tensor.matmul(seg, wih1[:, j * H:(j + 1) * H], xs,
                                     start=True, stop=False)
                    nc.tensor.matmul(
                        seg, wp[:, C_WHH1 + j * H:C_WHH1 + (j + 1) * H], h1[:],
                        start=False, stop=True)
                cell(pg1, h1, c1, h1, 0)

                pg2 = ppool.tile([128, 4 * BL], f32, tag="pg")
                for j in range(4):
                    seg = pg2[:, j * BL:(j + 1) * BL]
                    nc.tensor.matmul(
                        seg, wp[:, C_WIH2 + j * H:C_WIH2 + (j + 1) * H], h1[:],
                        start=True, stop=False)
                    nc.tensor.matmul(
                        seg, wp[:, C_WHH2 + j * H:C_WHH2 + (j + 1) * H], h2[:],
                        start=False, stop=True)
                cell(pg2, h2, c2, h2, 4)

            # ---- KDN head ----
            x_last = xbig[64:128, (T // 2 - 1) * BL:(T // 2) * BL]
            wk0x = wp[64:128, C_WK0X:C_WK0X + 128]
            pk = ppool.tile([128, 4 * BL], f32, tag="pg")
            seg = pk[:, 0:BL]
            nc.tensor.matmul(seg, wp[:, C_WK0H:C_WK0H + 128], h2[:],
                             start=True, stop=False)
            nc.tensor.matmul(seg, wk0x, x_last, start=False, stop=True)
            yk = gpool.tile([128, BL], bf16, tag="yk0")
            if zero_bias:
                nc.scalar.activation(yk[:], seg, AF.Relu)
            else:
                nc.scalar.activation(yk[:], seg, AF.Relu, bias=bpt[:, 8:9])
            for k in range(1, 5):
                pk2 = ppool.tile([128, 4 * BL], f32, tag="pg")
                seg = pk2[:, 0:BL]
                nc.tensor.matmul(
                    seg, wp[:, C_WK + (k - 1) * 128:C_WK + k * 128], yk[:],
                    start=True, stop=True)
                yk2 = gpool.tile([128, BL], bf16, tag=f"yk{k}")
                if zero_bias:
                    nc.scalar.activation(yk2[:], seg, AF.Relu)
                else:
                    nc.scalar.activation(yk2[:], seg, AF.Relu,
                                         bias=bpt[:, 8 + k:9 + k])
                yk = yk2
            pk5 = ppool.tile([128, 4 * BL], f32, tag="pg")
            seg5 = pk5[0:O, 0:BL]
            nc.tensor.matmul(seg5, wp[:, C_WK5:C_WK5 + O], yk[:],
                             start=True, stop=True)
            y5 = gpool.tile([128, BL], f32, tag="y5")
            if zero_bias:
                nc.scalar.copy(y5[0:O, :], seg5)
            else:
                nc.scalar.activation(y5[0:O, :], seg5, AF.Identity,
                                     bias=bpt[0:O, 13:14])

            nc.sync.dma_start(yT[:], y5[0:O, :])
            nc.sync.dma_start(h2T[:], h2[:])

    nc.compile()
    return nc


def _reorder_gates(a):
    """torch gate order (i,f,g,o) -> kernel order (g,o,i,f) along axis 0.

    tanh(g) runs first; sigmoid(i,f) feeds the c-update; sigmoid(o) is
    deferred past the c-update since h needs it last."""
    i, f, g, o = np.split(a, 4, axis=0)
    return np.concatenate([g, i, f, o], axis=0)


def _pack_weights(inp, zero_bias=False):
    def lstm_w(name):
        w = _reorder_gates(inp[name]).astype(np.float32).copy()
        if zero_bias:
            w[:H] *= 2.0  # fast path: tanh(g) = 2*sigmoid(2*g_pre) - 1
        return w.T

    Wp = np.zeros((128, WC), np.float32)
    wih1T = lstm_w("W_ih1")  # [64, 512]
    Wp[0:64, C_WIH1:C_WIH1 + 512] = wih1T
    Wp[64:128, C_WIH1:C_WIH1 + 512] = wih1T
    Wp[:, C_WHH1:C_WHH1 + 512] = lstm_w("W_hh1")
    Wp[:, C_WIH2:C_WIH2 + 512] = lstm_w("W_ih2")
    Wp[:, C_WHH2:C_WHH2 + 512] = lstm_w("W_hh2")
    Wp[:, C_WK0H:C_WK0H + 128] = inp["Wk0"][:, :H].T
    Wp[64:128, C_WK0X:C_WK0X + 128] = inp["Wk0"][:, H:].T
    for k in range(1, 5):
        Wp[:, C_WK + (k - 1) * 128:C_WK + k * 128] = inp[f"Wk{k}"].T
    Wp[:, C_WK5:C_WK5 + O] = inp["Wk5"].T
    bp = np.zeros((128, NB), np.float32)
    b1 = _reorder_gates((inp["b_ih1"] + inp["b_hh1"]).reshape(4 * H, 1))[:, 0]
    b2 = _reorder_gates((inp["b_ih2"] + inp["b_hh2"]).reshape(4 * H, 1))[:, 0]
    for j in range(4):
        bp[:, j] = b1[j * H:(j + 1) * H]
        bp[:, 4 + j] = b2[j * H:(j + 1) * H]
    for k in range(5):
        bp[:, 8 + k] = inp[f"bk{k}"]
    bp[0:O, 13] = inp["bk5"]
    return Wp, bp


_NC_CACHE = {}


def _get_nc(zero_bias):
    key = ("nc", bool(zero_bias))
    if key not in _NC_CACHE:
        _NC_CACHE[key] = _build_nc(zero_bias=zero_bias)
    return _NC_CACHE[key]


def run(inputs, trace=False, tmpdir=None):
    inp = {k: np.asarray(v) for k, v in inputs.items()}
    zero_bias = all(
        not np.any(np.asarray(inp[k]))
        for k in ("b_ih1", "b_hh1", "b_ih2", "b_hh2",
                  "bk0", "bk1", "bk2", "bk3", "bk4", "bk5"))
    Wp, bp = _pack_weights(inp, zero_bias=zero_bias)
    Wpb = Wp.astype(ml_dtypes.bfloat16)
    x = np.asarray(inp["x"], np.float32)
    in_maps = []
    for c in range(NCORES):
        xc = x[c * BL:(c + 1) * BL]                     # [BL, T, I]
        xt = np.ascontiguousarray(xc.transpose(1, 2, 0))  # [T, I, BL]
        xt = np.ascontiguousarray(
            xt.reshape(T // 2, 2, I, BL).transpose(1, 2, 0, 3)
        ).reshape(128, T // 2, BL).astype(ml_dtypes.bfloat16)
        in_maps.append({"xT": xt, "Wp": Wpb, "bp": bp})
    nc = _get_nc(zero_bias)
    r = run_bass_kernel_spmd(nc, in_maps, list(range(NCORES)),
                             trace=trace, tmpdir=tmpdir)
    ys, h2s = [], []
    for c in range(NCORES):
        res = r.results[c]
        ys.append(res["yT"].T)
        h2s.append(res["h2T"].astype(np.float32).T)
    y = np.ascontiguousarray(np.concatenate(ys)).astype(np.float32)
    h2 = np.ascontiguousarray(np.concatenate(h2s)).astype(np.float32)
    rr = np.ascontiguousarray(
        np.concatenate([h2, x[:, T - 1, :]], axis=1)).astype(np.float32)
    return (y, h2, rr), r


def kernel(**inputs):
    out, _ = run(inputs)
    return out


# revision 17
# speedup vs baseline: 1.0116x; 1.0053x over previous
"""DFMNET (2-layer LSTM + KDN MLP head) Trainium2 Bass kernel.

Data-parallel over batch: B=2048 split across 8 NeuronCores (256 rows each).
Per-core layout: features on partitions, batch on the free dimension.
  - states: h1,h2 [H=128, B=256] bf16 SBUF (matmul rhs must be SBUF);
    cell states c1,c2 [128, 256] f32 kept in PSUM so tanh(c) reads
    PSUM-source (ScalarE PSUM reads are ~2x faster than SBUF reads)
  - gates computed as gates^T [4H, B] f32 in PSUM via PE matmuls
    (x-part K=64 + h-part K=128 accumulated per gate), weights/x/h in
    bf16 (fp32 matmuls run as two hi/lo passes - 2x slower)
  - gate order repacked host-side to (g, i, f, o): tanh(g) issues first,
    sigmoid(i,f) as one 512-wide op feeds the c-update, sigmoid(o) is
    deferred past the c-update since h = o*tanh(c) needs it last
  - when all biases are zero (true for this model) activations fuse
    wide with no bias APs; otherwise a per-gate bias-AP path is used
  - x transposed host-side into [p=(t%2)*64+i, t//2, b] so it DMAs
    contiguously and serves directly as matmul rhs; W_ih1/Wk0x are
    duplicated into both 64-partition halves so even/odd timesteps hit
    matching partition ranges
"""
import sys

sys.path.insert(0, "/opt/trn_rl_repo")

import numpy as np

B, T, I = 2048, 256, 64
H = 128
K = 128
O = 64
NCORES = 8
BL = B // NCORES  # 256 batch rows per core

F32 = None  # set in _build


def _install_axon_hooks():
    """Provide antenv.axon_hooks (missing in the agent image) so trace=True
    can drive NTFF profiling. Harmless if profiling is never requested."""
    import contextlib
    import ctypes
    import types

    if "antenv.axon_hooks" in sys.modules:
        return
    so_path = "/opt/axon/libaxon_pjrt.so"

    def _make():
        try:
            lib = ctypes.CDLL(so_path)
        except OSError:
            return None
        if not hasattr(lib, "axon_start_nrt_profile"):
            return None
        lib.axon_start_nrt_profile.argtypes = [
            ctypes.POINTER(ctypes.c_int64),
            ctypes.c_size_t,
        ]
        lib.axon_start_nrt_profile.restype = ctypes.c_int64
        lib.axon_stop_nrt_profile.argtypes = [ctypes.c_char_p]
        lib.axon_stop_nrt_profile.restype = ctypes.c_int64

        @contextlib.contextmanager
        def _hook(output_dir, device_ids):
            import jax

            jax.devices()
            if device_ids:
                ids = (ctypes.c_int64 * len(device_ids))(*device_ids)
                rc = lib.axon_start_nrt_profile(ids, len(device_ids))
            else:
                rc = lib.axon_start_nrt_profile(None, 0)
            if rc != 0:
                raise RuntimeError(f"axon_start_nrt_profile rc={rc}")
            try:
                yield
            finally:
                n = lib.axon_stop_nrt_profile(str(output_dir).encode())
                if n < 0:
                    raise RuntimeError(f"axon_stop_nrt_profile rc={n}")

        return _hook

    hook = _make()
    mod = types.ModuleType("antenv.axon_hooks")
    mod.get_axon_ntff_profile_hook = lambda: hook
    mod.set_axon_ntff_profile_hook = lambda h: None
    sys.modules["antenv.axon_hooks"] = mod


_install_axon_hooks()

import ml_dtypes  # noqa: E402
import concourse.bacc as bacc  # noqa: E402
import concourse.tile as tile  # noqa: E402
import concourse.mybir as mybir  # noqa: E402
from concourse.bass_utils import run_bass_kernel_spmd  # noqa: E402

AF = mybir.ActivationFunctionType

# Weight-pack column layout (one [128, WC] f32 array, replicated per core)
C_WIH1 = 0        # [128, 512]  W_ih1T duplicated in rows 0-63 and 64-127
C_WHH1 = 512      # [128, 512]
C_WIH2 = 1024     # [128, 512]
C_WHH2 = 1536     # [128, 512]
C_WK0H = 2048     # [128, 128]
C_WK0X = 2176     # [128, 128]  rows 64-127 hold Wk0[:,128:].T
C_WK = 2304       # 4 x [128, 128]  Wk1..Wk4 transposed
C_WK5 = 2816      # [128, 64]
WC = 2880

NB = 14  # bias pack cols: 0-3 b1(i,f,o,g), 4-7 b2, 8-12 bk0..bk4, 13 bk5


def _build_nc(zero_bias=True):
    nc = bacc.Bacc("TRN2", target_bir_lowering=False, debug=False)
    f32 = mybir.dt.float32
    bf16 = mybir.dt.bfloat16

    xT = nc.dram_tensor("xT", [128, T // 2, BL], bf16, kind="ExternalInput")
    Wp = nc.dram_tensor("Wp", [128, WC], bf16, kind="ExternalInput")
    bp = nc.dram_tensor("bp", [128, NB], f32, kind="ExternalInput")
    yT = nc.dram_tensor("yT", [O, BL], f32, kind="ExternalOutput")
    h2T = nc.dram_tensor("h2T", [H, BL], bf16, kind="ExternalOutput")

    with tile.TileContext(nc) as tc:
        with (
            tc.tile_pool(name="const", bufs=1) as cpool,
            tc.tile_pool(name="state", bufs=1) as spool,
            tc.tile_pool(name="gates", bufs=3) as gpool,
            tc.tile_pool(name="cpsum", bufs=1, space="PSUM") as cppool,
            tc.tile_pool(name="psum", bufs=3, space="PSUM") as ppool,
        ):
            xbig = cpool.tile([128, (T // 2) * BL], bf16, tag="xbig")
            for c in range(8):
                nc.sync.dma_start(
                    xbig[:, c * 16 * BL:(c + 1) * 16 * BL],
                    xT[:, c * 16:(c + 1) * 16, :],
                )
            wp = cpool.tile([128, WC], bf16, tag="wp")
            nc.sync.dma_start(wp[:], Wp[:])
            bpt = cpool.tile([128, NB], f32, tag="bpt")
            nc.sync.dma_start(bpt[:], bp[:])

            h1 = spool.tile([128, BL], bf16, tag="h1")
            h2 = spool.tile([128, BL], bf16, tag="h2")
            c1 = cppool.tile([128, BL], f32, tag="c1")
            c2 = cppool.tile([128, BL], f32, tag="c2")
            nc.vector.memset(h1[:], 0.0)
            nc.vector.memset(h2[:], 0.0)
            nc.vector.memset(c1[:], 0.0)
            nc.vector.memset(c2[:], 0.0)

            # gate order in pack/psum: (g, i, f, o)
            def cell(pg, ccur, hnext, bias_base):
                gg = gpool.tile([128, BL], f32, tag="gg")
                if zero_bias:
                    # gate-g weights pre-scaled x2 host-side: all 4 gates
                    # take sigmoid in ONE 1024-wide ACT (tanh(x) =
                    # 2*sigmoid(2x)-1), g fixed up via DVE tensor_scalar
                    # Two bank-aligned sigmoids instead of one 1024-wide:
                    # bank0 (g,i) unblocks after its 2 h-part matmuls and
                    # feeds g_fix/ig on DVE while bank1 (f,o) still runs.
                    sg = gpool.tile([128, 4 * BL], f32, tag="sg")
                    nc.scalar.activation(sg[:, 0:2 * BL], pg[:, 0:2 * BL],
                                         AF.Sigmoid)
                    nc.scalar.activation(sg[:, 2 * BL:4 * BL],
                                         pg[:, 2 * BL:4 * BL], AF.Sigmoid)
                    nc.vector.tensor_scalar(
                        gg[:], sg[:, 0:BL], 2.0, 1.0,
                        mybir.AluOpType.mult, mybir.AluOpType.subtract)
                    ii, ff, oo = (sg[:, BL:2 * BL], sg[:, 2 * BL:3 * BL],
                                  sg[:, 3 * BL:4 * BL])
                    ig = gpool.tile([128, BL], f32, tag="ig")
                    nc.vector.tensor_mul(ig[:], ii, gg[:])
                    fc = gpool.tile([128, BL], f32, tag="fc")
                    nc.vector.tensor_mul(fc[:], ff, ccur[:])
                    nc.vector.tensor_add(ccur[:], fc[:], ig[:])
                    tc_ = gpool.tile([128, BL], bf16, tag="tc_")
                    nc.scalar.activation(tc_[:], ccur[:], AF.Tanh)
                    nc.vector.tensor_mul(hnext[:], oo, tc_[:])
                    return
                iit = gpool.tile([128, BL], f32, tag="ii")
                fft = gpool.tile([128, BL], f32, tag="ff")
                oot = gpool.tile([128, BL], f32, tag="oo")
                nc.scalar.activation(gg[:], pg[:, 0:BL], AF.Tanh,
                                     bias=bpt[:, bias_base:bias_base + 1])
                nc.scalar.activation(iit[:], pg[:, BL:2 * BL], AF.Sigmoid,
                                     bias=bpt[:, bias_base + 1:bias_base + 2])
                nc.scalar.activation(fft[:], pg[:, 2 * BL:3 * BL], AF.Sigmoid,
                                     bias=bpt[:, bias_base + 2:bias_base + 3])
                nc.scalar.activation(oot[:], pg[:, 3 * BL:4 * BL], AF.Sigmoid,
                                     bias=bpt[:, bias_base + 3:bias_base + 4])
                ii, ff, oo = iit[:], fft[:], oot[:]
                fc = gpool.tile([128, BL], f32, tag="fc")
                nc.vector.tensor_mul(fc[:], ff, ccur[:])
                ig = gpool.tile([128, BL], f32, tag="ig")
                nc.vector.tensor_mul(ig[:], ii, gg[:])
                nc.vector.tensor_add(ccur[:], fc[:], ig[:])
                tc_ = gpool.tile([128, BL], bf16, tag="tc_")
                nc.scalar.activation(tc_[:], ccur[:], AF.Tanh)
                nc.vector.tensor_mul(hnext[:], oo, tc_[:])

            for t in range(T):
                tp, t2 = t % 2, t // 2
                xs = xbig[tp * 64:(tp + 1) * 64, t2 * BL:(t2 + 1) * BL]
                wih1 = wp[tp * 64:(tp + 1) * 64, C_WIH1:C_WIH1 + 512]

                pg1 = ppool.tile([128, 4 * BL], f32, tag="pg")
                for j in range(4):
                    seg = pg1[:, j * BL:(j + 1) * BL]
                    nc.tensor.matmul(seg, wih1[:, j * H:(j + 1) * H], xs,
                                     start=True, stop=False)
                    nc.tensor.matmul(
                        seg, wp[:, C_WHH1 + j * H:C_WHH1 + (j + 1) * H], h1[:],
                        start=False, stop=True)
                cell(pg1, h1, c1, h1, 0)

                pg2 = ppool.tile([128, 4 * BL], f32, tag="pg")
                for j in range(4):
                    seg = pg2[:, j * BL:(j + 1) * BL]
                    nc.tensor.matmul(
                        seg, wp[:, C_WIH2 + j * H:C_WIH2 + (j + 1) * H], h1[:],
                        start=True, stop=False)
                    nc.tensor.matmul(
                        seg, wp[:, C_WHH2 + j * H:C_WHH2 + (j + 1) * H], h2[:],
                        start=False, stop=True)
                cell(pg2, h2, c2, h2, 4)

            # ---- KDN head ----
            x_last = xbig[64:128, (T // 2 - 1) * BL:(T // 2) * BL]
            wk0x = wp[64:128, C_WK0X:C_WK0X + 128]
            pk = ppool.tile([128, 4 * BL], f32, tag="pg")
            seg = pk[:, 0:BL]
            nc.tensor.matmul(seg, wp[:, C_WK0H:C_WK0H + 128], h2[:],
                             start=True, stop=False)
            nc.tensor.matmul(seg, wk0x, x_last, start=False, stop=True)
            yk = gpool.tile([128, BL], bf16, tag="yk0")
            if zero_bias:
                nc.scalar.activation(yk[:], seg, AF.Relu)
            else:
                nc.scalar.activation(yk[:], seg, AF.Relu, bias=bpt[:, 8:9])
            for k in range(1, 5):
                pk2 = ppool.tile([128, 4 * BL], f32, tag="pg")
                seg = pk2[:, 0:BL]
                nc.tensor.matmul(
                    seg, wp[:, C_WK + (k - 1) * 128:C_WK + k * 128], yk[:],
                    start=True, stop=True)
                yk2 = gpool.tile([128, BL], bf16, tag=f"yk{k}")
                if zero_bias:
                    nc.scalar.activation(yk2[:], seg, AF.Relu)
                else:
                    nc.scalar.activation(yk2[:], seg, AF.Relu,
                                         bias=bpt[:, 8 + k:9 + k])
                yk = yk2
            pk5 = ppool.tile([128, 4 * BL], f32, tag="pg")
            seg5 = pk5[0:O, 0:BL]
            nc.tensor.matmul(seg5, wp[:, C_WK5:C_WK5 + O], yk[:],
                             start=True, stop=True)
            y5 = gpool.tile([128, BL], f32, tag="y5")
            if zero_bias:
                nc.scalar.copy(y5[0:O, :], seg5)
            else:
                nc.scalar.activation(y5[0:O, :], seg5, AF.Identity,
                                     bias=bpt[0:O, 13:14])

            nc.sync.dma_start(yT[:], y5[0:O, :])
            nc.sync.dma_start(h2T[:], h2[:])

    nc.compile()
    return nc


def _reorder_gates(a):
    """torch gate order (i,f,g,o) -> kernel order (g,o,i,f) along axis 0.

    tanh(g) runs first; sigmoid(i,f) feeds the c-update; sigmoid(o) is
    deferred past the c-update since h needs it last."""
    i, f, g, o = np.split(a, 4, axis=0)
    return np.concatenate([g, i, f, o], axis=0)


def _pack_weights(inp, zero_bias=False):
    def lstm_w(name):
        w = _reorder_gates(inp[name]).astype(np.float32).copy()
        if zero_bias:
            w[:H] *= 2.0  # fast path: tanh(g) = 2*sigmoid(2*g_pre) - 1
        return w.T

    Wp = np.zeros((128, WC), np.float32)
    wih1T = lstm_w("W_ih1")  # [64, 512]
    Wp[0:64, C_WIH1:C_WIH1 + 512] = wih1T
    Wp[64:128, C_WIH1:C_WIH1 + 512] = wih1T
    Wp[:, C_WHH1:C_WHH1 + 512] = lstm_w("W_hh1")
    Wp[:, C_WIH2:C_WIH2 + 512] = lstm_w("W_ih2")
    Wp[:, C_WHH2:C_WHH2 + 512] = lstm_w("W_hh2")
    Wp[:, C_WK0H:C_WK0H + 128] = inp["Wk0"][:, :H].T
    Wp[64:128, C_WK0X:C_WK0X + 128] = inp["Wk0"][:, H:].T
    for k in range(1, 5):
        Wp[:, C_WK + (k - 1) * 128:C_WK + k * 128] = inp[f"Wk{k}"].T
    Wp[:, C_WK5:C_WK5 + O] = inp["Wk5"].T
    bp = np.zeros((128, NB), np.float32)
    b1 = _reorder_gates((inp["b_ih1"] + inp["b_hh1"]).reshape(4 * H, 1))[:, 0]
    b2 = _reorder_gates((inp["b_ih2"] + inp["b_hh2"]).reshape(4 * H, 1))[:, 0]
    for j in range(4):
        bp[:, j] = b1[j * H:(j + 1) * H]
        bp[:, 4 + j] = b2[j * H:(j + 1) * H]
    for k in range(5):
        bp[:, 8 + k] = inp[f"bk{k}"]
    bp[0:O, 13] = inp["bk5"]
    return Wp, bp


_NC_CACHE = {}


def _get_nc(zero_bias):
    key = ("nc", bool(zero_bias))
    if key not in _NC_CACHE:
        _NC_CACHE[key] = _build_nc(zero_bias=zero_bias)
    return _NC_CACHE[key]


def run(inputs, trace=False, tmpdir=None):
    inp = {k: np.asarray(v) for k, v in inputs.items()}
    zero_bias = all(
        not np.any(np.asarray(inp[k]))
        for k in ("b_ih1", "b_hh1", "b_ih2", "b_hh2",
                  "bk0", "bk1", "bk2", "bk3", "bk4", "bk5"))
    Wp, bp = _pack_weights(inp, zero_bias=zero_bias)
    Wpb = Wp.astype(ml_dtypes.bfloat16)
    x = np.asarray(inp["x"], np.float32)
    in_maps = []
    for c in range(NCORES):
        xc = x[c * BL:(c + 1) * BL]                     # [BL, T, I]
        xt = np.ascontiguousarray(xc.transpose(1, 2, 0))  # [T, I, BL]
        xt = np.ascontiguousarray(
            xt.reshape(T // 2, 2, I, BL).transpose(1, 2, 0, 3)
        ).reshape(128, T // 2, BL).astype(ml_dtypes.bfloat16)
        in_maps.append({"xT": xt, "Wp": Wpb, "bp": bp})
    nc = _get_nc(zero_bias)
    r = run_bass_kernel_spmd(nc, in_maps, list(range(NCORES)),
                             trace=trace, tmpdir=tmpdir)
    ys, h2s = [], []
    for c in range(NCORES):
        res = r.results[c]
        ys.append(res["yT"].T)
        h2s.append(res["h2T"].astype(np.float32).T)
    y = np.ascontiguousarray(np.concatenate(ys)).astype(np.float32)
    h2 = np.ascontiguousarray(np.concatenate(h2s)).astype(np.float32)
    rr = np.ascontiguousarray(
        np.concatenate([h2, x[:, T - 1, :]], axis=1)).astype(np.float32)
    return (y, h2, rr), r


def kernel(**inputs):
    out, _ = run(inputs)
    return out
